# revision 3
# baseline (speedup 1.0000x reference)
"""Trainium2 Bass kernel for the coverage-attention module.

Self-contained: hardcodes shapes B=128, S=512, D=1024, 8 NeuronCores,
data-parallel over batch (16 rows per core).

Per-core dataflow (all matmuls bf16 operands, fp32 PSUM accumulation):
  att.T[e_tile, s]  = sum_dt Wh.T[dt, e_tile].T @ enc.T[dt, s]   (PE)
                      + wc[e_tile] (x) cov[s]                    (K=1 aug MM)
  tanh via ACT with per-partition bias = dec_feat.T[:, b]        (ACT)
  score[1, s]       = sum_et v[e_tile].T @ tanh[e_tile, s]       (PE, M=1)
  softmax on a single partition row; the reference's
  softmax->mask->renormalize collapses to e*mask/sum(e*mask)     (DVE)
  aw broadcast to 128 partitions via ones-outer-product matmul   (PE)
  context.T[dt, b]  = reduce_s(enc.T[dt, s] * aw_bcast)          (DVE TTR)
  context transposed back via PE transpose at the end.
"""

import os
from contextlib import ExitStack

import numpy as np
import ml_dtypes

from concourse import bacc, tile, mybir
from concourse.bass_utils import run_bass_kernel_spmd

B, S, H = 128, 512, 512
D = 2 * H          # 1024
NCORES = 8
BL = B // NCORES   # 16 batch rows per core
NT = D // 128      # 8 tiles of 128 along D

BF = mybir.dt.bfloat16
F32 = mybir.dt.float32
bf16 = ml_dtypes.bfloat16

_CACHE = {}


def _build():
    nc = bacc.Bacc("TRN2", target_bir_lowering=False, debug=False,
                   num_devices=NCORES)

    encT = nc.dram_tensor("encT", [BL, D, S], BF, kind="ExternalInput").ap()
    whT = nc.dram_tensor("whT", [D, D], BF, kind="ExternalInput").ap()
    wsT = nc.dram_tensor("wsT", [D, D], BF, kind="ExternalInput").ap()
    stT = nc.dram_tensor("stT", [128, NT * BL], BF, kind="ExternalInput").ap()
    wsb = nc.dram_tensor("wsb", [128, NT], F32, kind="ExternalInput").ap()
    vT = nc.dram_tensor("vT", [128, NT], BF, kind="ExternalInput").ap()
    wc = nc.dram_tensor("wc", [1, D], BF, kind="ExternalInput").ap()
    covb = nc.dram_tensor("covb", [1, BL * S], BF, kind="ExternalInput").ap()
    covf = nc.dram_tensor("covf", [1, BL * S], F32, kind="ExternalInput").ap()
    maskf = nc.dram_tensor("maskf", [1, BL * S], F32, kind="ExternalInput").ap()
    ident = nc.dram_tensor("ident", [128, 128], F32, kind="ExternalInput").ap()

    ctx_o = nc.dram_tensor("ctx", [BL, D], F32, kind="ExternalOutput").ap()
    aw_o = nc.dram_tensor("aw", [BL, S], F32, kind="ExternalOutput").ap()
    ncov_o = nc.dram_tensor("ncov", [BL, S], F32, kind="ExternalOutput").ap()

    AF = mybir.ActivationFunctionType
    ALU = mybir.AluOpType
    AX = mybir.AxisListType

    with tile.TileContext(nc) as tc, ExitStack() as es:
        cp = es.enter_context(tc.tile_pool(name="const", bufs=1))

        wht_sb = []
        wst_sb = []
        for t in range(NT):
            w1 = cp.tile([128, D], BF, tag=f"wht{t}")
            nc.sync.dma_start(w1[:], whT[t * 128:(t + 1) * 128, :])
            wht_sb.append(w1)
            w2 = cp.tile([128, D], BF, tag=f"wst{t}")
            nc.sync.dma_start(w2[:], wsT[t * 128:(t + 1) * 128, :])
            wst_sb.append(w2)

        stT_sb = cp.tile([128, NT * BL], BF, tag="stT")
        nc.sync.dma_start(stT_sb[:], stT[:])
        wsb_sb = cp.tile([128, NT], F32, tag="wsb")
        nc.sync.dma_start(wsb_sb[:], wsb[:])
        vT_sb = cp.tile([128, NT], BF, tag="vT")
        nc.sync.dma_start(vT_sb[:], vT[:])
        wc_sb = cp.tile([1, D], BF, tag="wc")
        nc.sync.dma_start(wc_sb[:], wc[:])
        covb_sb = cp.tile([1, BL * S], BF, tag="covb")
        nc.sync.dma_start(covb_sb[:], covb[:])
        covf_sb = cp.tile([1, BL * S], F32, tag="covf")
        nc.sync.dma_start(covf_sb[:], covf[:])
        mask_sb = cp.tile([1, BL * S], F32, tag="mask")
        nc.sync.dma_start(mask_sb[:], maskf[:])
        id_sb = cp.tile([128, 128], F32, tag="ident")
        nc.sync.dma_start(id_sb[:], ident[:])
        ones_sb = cp.tile([1, 128], BF, tag="ones")
        nc.gpsimd.memset(ones_sb[:], 1.0)

        dec_sb = [cp.tile([128, BL], F32, tag=f"dec{t}", name=f"dec{t}")
                  for t in range(NT)]
        ctx_cols = [cp.tile([128, BL], F32, tag=f"ctxc{t}", name=f"ctxc{t}")
                    for t in range(NT)]
        ctx_out = cp.tile([BL, D], F32, tag="ctxout")

        # phase 0: dec_feat.T[e, b] = Ws.T-tiles.T @ s_t.T, + Ws_b via ACT bias
        with tc.tile_pool(name="ph0", bufs=2, space="PSUM") as ph0:
            for et in range(NT):
                dp = ph0.tile([128, BL], F32, tag="decp")
                for dt in range(NT):
                    nc.tensor.matmul(
                        dp[:],
                        wst_sb[dt][:, et * 128:(et + 1) * 128],
                        stT_sb[:, dt * BL:(dt + 1) * BL],
                        start=(dt == 0), stop=(dt == NT - 1),
                    )
                nc.scalar.activation(dec_sb[et][:], dp[:], AF.Identity,
                                     bias=wsb_sb[:, et:et + 1])

        encp = es.enter_context(tc.tile_pool(name="enc", bufs=3))
        tanhp = es.enter_context(tc.tile_pool(name="tanh", bufs=3))
        attp = es.enter_context(tc.tile_pool(name="attps", bufs=2, space="PSUM"))
        scorep = es.enter_context(tc.tile_pool(name="scoreps", bufs=2, space="PSUM"))
        bcp = es.enter_context(tc.tile_pool(name="bcps", bufs=2, space="PSUM"))
        trp = es.enter_context(tc.tile_pool(name="trps", bufs=2, space="PSUM"))
        smp = es.enter_context(tc.tile_pool(name="sm", bufs=2))
        awbcp = es.enter_context(tc.tile_pool(name="awbc", bufs=2))
        tmpp = es.enter_context(tc.tile_pool(name="ctmp", bufs=2))

        def emit_load(b):
            et_sb = encp.tile([128, NT * S], BF, tag="enc")
            for dt in range(NT):
                nc.sync.dma_start(et_sb[:, dt * S:(dt + 1) * S],
                                  encT[b, dt * 128:(dt + 1) * 128, :])
            return et_sb

        def emit_att_score(b, et_sb):
            sc = scorep.tile([1, S], F32, tag="score")
            prev = None
            for et in range(NT):
                ap_ = attp.tile([128, S], F32, tag="att")
                nc.tensor.matmul(
                    ap_[:],
                    wc_sb[0:1, et * 128:(et + 1) * 128],
                    covb_sb[0:1, b * S:(b + 1) * S],
                    start=True, stop=False,
                )
                for dt in range(NT):
                    nc.tensor.matmul(
                        ap_[:],
                        wht_sb[dt][:, et * 128:(et + 1) * 128],
                        et_sb[:, dt * S:(dt + 1) * S],
                        start=False, stop=(dt == NT - 1),
                    )
                th = tanhp.tile([128, S], BF, tag="tanh")
                nc.scalar.activation(th[:], ap_[:], AF.Tanh,
                                     bias=dec_sb[et][:, b:b + 1])
                if prev is not None:
                    pet, pth = prev
                    nc.tensor.matmul(sc[:], vT_sb[:, pet:pet + 1], pth[:],
                                     start=(pet == 0), stop=False)
                prev = (et, th)
            pet, pth = prev
            nc.tensor.matmul(sc[:], vT_sb[:, pet:pet + 1], pth[:],
                             start=False, stop=True)
            return sc

        def emit_post(b, et_sb, sc):
            exp_sb = smp.tile([1, S], F32, tag="exp")
            nc.scalar.activation(exp_sb[:], sc[:], AF.Exp)
            em = smp.tile([1, S], F32, tag="emask")
            nc.vector.tensor_mul(em[:], exp_sb[:], mask_sb[0:1, b * S:(b + 1) * S])
            dn = smp.tile([1, 1], F32, tag="dn")
            nc.vector.reduce_sum(dn[:], em[:], axis=AX.X)
            rd = smp.tile([1, 1], F32, tag="rd")
            nc.vector.reciprocal(rd[:], dn[:])
            awr = smp.tile([1, S], F32, tag="awr")
            nc.vector.tensor_scalar_mul(awr[:], em[:], rd[:])
            ncv = smp.tile([1, S], F32, tag="ncv")
            nc.vector.tensor_add(ncv[:], awr[:], covf_sb[0:1, b * S:(b + 1) * S])
            nc.sync.dma_start(aw_o[b:b + 1, :], awr[:])
            nc.sync.dma_start(ncov_o[b:b + 1, :], ncv[:])
            awb = smp.tile([1, S], BF, tag="awb")
            nc.vector.tensor_copy(awb[:], awr[:])
            bc = bcp.tile([128, S], F32, tag="bc")
            nc.tensor.matmul(bc[:], ones_sb[:], awb[:], start=True, stop=True)
            awbc = awbcp.tile([128, S], BF, tag="awbc")
            nc.scalar.copy(awbc[:], bc[:])
            # note: tensor_tensor_reduce wedges the device here (HW-only
            # failure; CoreSim accepts it) — use separate mul + reduce.
            for dt in range(NT):
                tmp = tmpp.tile([128, S], BF, tag="tmp")
                nc.vector.tensor_mul(tmp[:], et_sb[:, dt * S:(dt + 1) * S],
                                     awbc[:])
                nc.vector.reduce_sum(ctx_cols[dt][:, b:b + 1], tmp[:],
                                     axis=AX.X)

        prev_state = None
        for b in range(BL):
            et_sb = emit_load(b)
            sc = emit_att_score(b, et_sb)
            if prev_state is not None:
                emit_post(*prev_state)
            prev_state = (b, et_sb, sc)
        emit_post(*prev_state)

        for dt in range(NT):
            tr = trp.tile([BL, 128], F32, tag="tr")
            nc.tensor.transpose(tr[:], ctx_cols[dt][:], id_sb[:])
            nc.scalar.copy(ctx_out[:, dt * 128:(dt + 1) * 128], tr[:])
        nc.sync.dma_start(ctx_o[:, :], ctx_out[:])

    nc.compile()
    return nc


def get_nc():
    if "nc" not in _CACHE:
        _CACHE["nc"] = _build()
    return _CACHE["nc"]


def make_in_maps(h_dec, c_dec, encoder_output, x_padding_masks, coverage_vector,
                 Wh_w, Ws_w, Ws_b, v_w, wc_w):
    h_dec = np.asarray(h_dec, np.float32)
    c_dec = np.asarray(c_dec, np.float32)
    encoder_output = np.asarray(encoder_output, np.float32)
    x_padding_masks = np.asarray(x_padding_masks, np.float32)
    coverage_vector = np.asarray(coverage_vector, np.float32)
    Wh_w = np.asarray(Wh_w, np.float32)
    Ws_w = np.asarray(Ws_w, np.float32)
    Ws_b = np.asarray(Ws_b, np.float32)
    v_w = np.asarray(v_w, np.float32)
    wc_w = np.asarray(wc_w, np.float32)

    whT = np.ascontiguousarray(Wh_w.T).astype(bf16)
    wsT = np.ascontiguousarray(Ws_w.T).astype(bf16)
    wsb = np.ascontiguousarray(Ws_b.reshape(NT, 128).T)
    vT = np.ascontiguousarray(v_w.reshape(NT, 128).T).astype(bf16)
    wc = wc_w.reshape(1, D).astype(bf16)
    ident = np.eye(128, dtype=np.float32)

    s_t = np.concatenate([h_dec[0], c_dec[0]], axis=1)  # (B, D)

    in_maps = []
    for c in range(NCORES):
        sl = slice(c * BL, (c + 1) * BL)
        encT = np.ascontiguousarray(
            encoder_output[sl].transpose(0, 2, 1)).astype(bf16)
        stT = np.ascontiguousarray(
            s_t[sl].T.reshape(NT, 128, BL).transpose(1, 0, 2).reshape(128, NT * BL)
        ).astype(bf16)
        covf = coverage_vector[sl].reshape(1, BL * S)
        in_maps.append({
            "encT": encT,
            "whT": whT,
            "wsT": wsT,
            "stT": stT,
            "wsb": wsb,
            "vT": vT,
            "wc": wc,
            "covb": covf.astype(bf16),
            "covf": np.ascontiguousarray(covf),
            "maskf": np.ascontiguousarray(
                x_padding_masks[sl].reshape(1, BL * S)),
            "ident": ident,
        })
    return in_maps


def kernel(**inputs):
    nc = get_nc()
    in_maps = make_in_maps(**inputs)
    res = run_bass_kernel_spmd(nc, in_maps, core_ids=list(range(NCORES)),
                               trace=False).results
    ctx = np.concatenate([res[c]["ctx"] for c in range(NCORES)], axis=0)
    aw = np.concatenate([res[c]["aw"] for c in range(NCORES)], axis=0)
    ncov = np.concatenate([res[c]["ncov"] for c in range(NCORES)], axis=0)
    return ctx, aw, ncov
